# revision 10
# baseline (speedup 1.0000x reference)
"""MoE (8 experts, top-2, d=1024, N=8192) on 8 trn2 NeuronCores.

Strategy (expert-parallel, per sharding hint):
 - Host computes routing (top-2 expert ids per token in fp64 for stable
   ordering) and dispatches: core e receives the tokens routed to expert e,
   pre-transposed as xgT [1024, C] (C = padded max expert load).
 - Device (per core, SPMD): router logits for its tokens (replicated router),
   top-2 gate g = sigmoid(2*l_own - m1 - m2), expert matmul
   y = (xg @ W[e] + b[e]) * g  with PSUM K-accumulation. Biases are added via
   DVE with host-pre-broadcast [128, .] bias tiles (K=1 fp32 matmuls fault on
   this HW path).
 - Host combines: out[idx_e] += y_e  (each token appears in exactly 2 experts'
   index lists; indices unique within an expert).
"""

import os
from contextlib import ExitStack

import numpy as np

import concourse.bass as bass
import concourse.bacc as bacc
import concourse.mybir as mybir
import concourse.tile as tile
from concourse.bass import ts
from concourse.bass_utils import run_bass_kernel_spmd

N_EXPERTS = 8
TOP_K = 2
D = 1024
N_CORES = 8
P = 128  # partitions
KT = D // P  # number of K tiles (8)
NH = 512  # psum free-dim tile (one bank of fp32)
E9 = N_EXPERTS + 2  # 8 experts + own-logit col + pad col (fp32r needs even)

# matmul operand dtype for the big expert matmul:
#   "f32"  : plain fp32 (4 cycles/row)
#   "f32r" : float32r (1 cycle/row at free dim >= 256)
MM_DTYPE = os.environ.get("MOE_MM_DTYPE", "f32")

LAST_RESULTS = None  # stash of BassKernelResults for test harness inspection

_BUILD_CACHE = {}


def _build(C: int, repeat: int = 1):
    """Build the SPMD Bass module for per-core padded token count C."""
    key = (C, MM_DTYPE, repeat)
    if key in _BUILD_CACHE:
        return _BUILD_CACHE[key]

    f32 = mybir.dt.float32
    mm_dt = mybir.dt.float32r if MM_DTYPE == "f32r" else mybir.dt.float32
    T = C // P

    nc = bacc.Bacc(None, target_bir_lowering=False)
    xgT = nc.declare_dram_parameter("xgT", [D, C], mm_dt, isOutput=False)
    w = nc.declare_dram_parameter("w", [D, D], mm_dt, isOutput=False)
    wr9 = nc.declare_dram_parameter("wr9", [D, E9], mm_dt, isOutput=False)
    # biases pre-broadcast to 128 partitions on host
    br9b = nc.declare_dram_parameter("br9b", [P, E9], f32, isOutput=False)
    bexpb = nc.declare_dram_parameter("bexpb", [P, D], f32, isOutput=False)
    y = nc.declare_dram_parameter("y", [C, D], f32, isOutput=True)

    with tile.TileContext(nc) as tc, ExitStack() as ctx:
        consts = ctx.enter_context(tc.tile_pool(name="consts", bufs=1))
        xpool = ctx.enter_context(tc.tile_pool(name="x", bufs=3))
        gpool = ctx.enter_context(tc.tile_pool(name="gates", bufs=3))
        ypool = ctx.enter_context(tc.tile_pool(name="y", bufs=3))
        lpsum = ctx.enter_context(
            tc.tile_pool(name="lpsum", bufs=2, space=bass.MemorySpace.PSUM)
        )
        ypsum = ctx.enter_context(
            tc.tile_pool(name="ypsum", bufs=4, space=bass.MemorySpace.PSUM)
        )

        # ---- constants / weights resident in SBUF ----
        w_sb = consts.tile([P, KT, D], mm_dt)
        nc.sync.dma_start(w_sb[:], w.rearrange("(kt p) n -> p kt n", p=P))

        wr9_sb = consts.tile([P, KT, E9], mm_dt)
        nc.sync.dma_start(wr9_sb[:], wr9.rearrange("(kt p) n -> p kt n", p=P))

        br9_sb = consts.tile([P, E9], f32)
        nc.sync.dma_start(br9_sb[:], br9b[:, :])

        b_sb = consts.tile([P, D], f32)
        nc.sync.dma_start(b_sb[:], bexpb[:, :])

        xgT_r = xgT.rearrange("(kt p) c -> p kt c", p=P)

        rep_cm = tc.For_i(0, repeat, 1) if repeat > 1 else None
        if rep_cm is not None:
            rep_cm.__enter__()
        if True:
            for t in range(T):
                # ---- load token tile (transposed): [128 din, 8 kt, 128 tok] ----
                xt = xpool.tile([P, KT, P], mm_dt)
                nc.sync.dma_start(xt[:], xgT_r[:, :, ts(t, P)])

                # ---- router logits: L[:, 0:8] full, L[:, 8] own expert ----
                Lp = lpsum.tile([P, E9], f32)
                for j in range(KT):
                    nc.tensor.matmul(
                        Lp[:], xt[:, j, :], wr9_sb[:, j, :],
                        start=(j == 0), stop=(j == KT - 1),
                    )
                La = gpool.tile([P, E9], f32)
                nc.vector.tensor_add(La[:], Lp[:], br9_sb[:])

                # ---- top-2 gate for own expert ----
                m1 = gpool.tile([P, 1], f32)
                nc.vector.reduce_max(
                    m1[:], La[:, 0:N_EXPERTS], axis=mybir.AxisListType.X
                )
                eq = gpool.tile([P, N_EXPERTS], f32)
                nc.vector.tensor_scalar(
                    eq[:], La[:, 0:N_EXPERTS], m1[:], None, mybir.AluOpType.is_equal
                )
                lm = gpool.tile([P, N_EXPERTS], f32)
                nc.vector.tensor_scalar_mul(lm[:], eq[:], -1e30)
                nc.vector.tensor_add(lm[:], lm[:], La[:, 0:N_EXPERTS])
                m2 = gpool.tile([P, 1], f32)
                nc.vector.reduce_max(m2[:], lm[:], axis=mybir.AxisListType.X)
                nsum = gpool.tile([P, 1], f32)
                nc.vector.tensor_add(nsum[:], m1[:], m2[:])
                nc.vector.tensor_scalar_mul(nsum[:], nsum[:], -1.0)
                g = gpool.tile([P, 1], f32)
                nc.scalar.activation(
                    g[:],
                    La[:, N_EXPERTS : N_EXPERTS + 1],
                    mybir.ActivationFunctionType.Sigmoid,
                    bias=nsum[:],
                    scale=2.0,
                )

                # ---- expert matmul + bias + gate scale ----
                for nh in range(D // NH):
                    yp = ypsum.tile([P, NH], f32)
                    for j in range(KT):
                        nc.tensor.matmul(
                            yp[:],
                            xt[:, j, :],
                            w_sb[:, j, ts(nh, NH)],
                            start=(j == 0),
                            stop=(j == KT - 1),
                        )
                    ysb = ypool.tile([P, NH], f32)
                    nc.vector.tensor_add(ysb[:], yp[:], b_sb[:, ts(nh, NH)])
                    nc.vector.tensor_scalar_mul(ysb[:], ysb[:], g[:])
                    nc.sync.dma_start(y[ts(t, P), ts(nh, NH)], ysb[:])
        if rep_cm is not None:
            rep_cm.__exit__(None, None, None)

    nc.compile()
    _BUILD_CACHE[key] = nc
    return nc


def _route(x, Wr, br):
    """Host routing in fp64: per-token top-2 expert ids."""
    n_tokens = x.shape[0]
    logits = x.astype(np.float64) @ Wr.astype(np.float64) + br.astype(np.float64)
    i1 = np.argmax(logits, axis=1)
    l2 = logits.copy()
    l2[np.arange(n_tokens), i1] = -np.inf
    i2 = np.argmax(l2, axis=1)
    return i1, i2


def _make_in_maps(x, Wr, br, W, b, idx_per_e, C):
    in_maps = []
    for e in range(N_CORES):
        idx = idx_per_e[e]
        xg = np.zeros((C, D), dtype=np.float32)
        xg[: len(idx)] = x[idx]
        xgT = np.ascontiguousarray(xg.T)
        zcol = np.zeros((D, 1), dtype=np.float32)
        wr9 = np.concatenate([Wr, Wr[:, e : e + 1], zcol], axis=1)
        br9 = np.concatenate([br, br[e : e + 1], np.zeros(1, np.float32)])[
            None, :
        ].astype(np.float32)
        in_maps.append(
            {
                "xgT": xgT,
                "w": np.ascontiguousarray(W[e]),
                "wr9": np.ascontiguousarray(wr9),
                "br9b": np.broadcast_to(br9, (P, E9)).copy(),
                "bexpb": np.broadcast_to(b[e][None, :], (P, D)).copy(),
            }
        )
    return in_maps


def _prep(inputs):
    x = np.asarray(inputs["x"], dtype=np.float32)
    Wr = np.asarray(inputs["Wr"], dtype=np.float32)
    br = np.asarray(inputs["br"], dtype=np.float32)
    W = np.asarray(inputs["W"], dtype=np.float32)
    b = np.asarray(inputs["b"], dtype=np.float32)
    i1, i2 = _route(x, Wr, br)
    idx_per_e = [np.where((i1 == e) | (i2 == e))[0] for e in range(N_EXPERTS)]
    C = max(P, ((max(len(ix) for ix in idx_per_e) + P - 1) // P) * P)
    in_maps = _make_in_maps(x, Wr, br, W, b, idx_per_e, C)
    return in_maps, idx_per_e, C, x.shape[0]


def kernel(**inputs) -> np.ndarray:
    global LAST_RESULTS
    in_maps, idx_per_e, C, n_tokens = _prep(inputs)
    nc = _build(C)
    res = run_bass_kernel_spmd(nc, in_maps, core_ids=list(range(N_CORES)))
    LAST_RESULTS = res

    out = np.zeros((n_tokens, D), dtype=np.float32)
    for e in range(N_CORES):
        idx = idx_per_e[e]
        out[idx] += res.results[e]["y"][: len(idx)]
    return out
